# revision 17
# baseline (speedup 1.0000x reference)
"""Trainium2 Bass kernel for nn_AutoregressiveEncoder (8 NeuronCores).

Model (see reference): per timestep t, a 2-layer bidirectional LSTM applied to
a length-1 sequence [x_t ; pred_{t-1}], hidden state carried across t, then a
linear head; the prediction feeds back into the next step's input.

Strategy: 8-way tensor parallel, gate-dim sliced (each core owns 64 h-units
per direction per layer = 256 of the 2048 gate rows per cell); batch (32) kept
whole. Per step, two AllGathers distribute the per-core h-slices. The pred
feedback is algebraically composed away: W0p @ pred = (W0p @ W_fc) @ h1 +
W0p @ b_fc, so the FC head (replicated, fed by gathered h1) only produces the
output tensor and is off the serial critical path.

Matmul layout ("weights moving"): activations live feature-major and serve as
the 128x32 stationary operand; weights stream as the [128, <=512] moving
operand in float32r (TF32-like, 1 cyc/row). Gates land batch-major [32, 512]
in PSUM where the LSTM cell elementwise runs. Gathered h tensors are kept in
the AllGather's natural j-major block order; weight rows are host-permuted to
match, and per-direction Whh blocks are zero-padded block-diagonal so one
N=512 matmul per gathered K-tile covers both directions.

Hardcoded shapes: B=32, T=512, IN=OUT=512, LH=512 (per-direction), L=2.
"""
import os
import numpy as np

import concourse.bacc as bacc
import concourse.mybir as mybir
import concourse.tile as tile
from concourse.bass_utils import run_bass_kernel_spmd

B, T_FULL, IN, OUT = 32, 512, 512, 512
LH = 512          # per-direction hidden size
NC = 8            # cores
S = LH // NC      # h-units per direction per core (64)
G = 4 * S         # gate rows per direction per core (256)

F32 = mybir.dt.float32
F32R = mybir.dt.float16  # compute dtype for PE-facing tensors
AFT = mybir.ActivationFunctionType

FAKE_AG = bool(int(os.environ.get("FAKE_AG", "0")))


def build(T=T_FULL):
    nc = bacc.Bacc(trn_type="TRN2", num_devices=NC)

    # ---- DRAM parameters (per core) ----
    # weight row order for h-contractions is the gathered j-major order:
    # row (j*128 + d*64 + i) <-> direction d, unit j*64+i
    xT = nc.declare_dram_parameter("xT", [T, IN, B], F32R, isOutput=False)
    w0x = nc.declare_dram_parameter("w0x", [IN, 2 * G], F32R, isOutput=False)
    w0h = nc.declare_dram_parameter("w0h", [2 * LH, 2 * G], F32R, isOutput=False)
    wcomb = nc.declare_dram_parameter("wcomb", [2 * LH, 2 * G], F32R, isOutput=False)
    w1x = nc.declare_dram_parameter("w1x", [2 * LH, 2 * G], F32R, isOutput=False)
    w1h = nc.declare_dram_parameter("w1h", [2 * LH, 2 * G], F32R, isOutput=False)
    wfc = nc.declare_dram_parameter("wfc", [2 * LH, OUT], F32R, isOutput=False)
    bias0 = nc.declare_dram_parameter("bias0", [1, 2 * G], F32R, isOutput=False)
    bias0z = nc.declare_dram_parameter("bias0z", [1, 2 * G], F32R, isOutput=False)
    bias1 = nc.declare_dram_parameter("bias1", [1, 2 * G], F32R, isOutput=False)
    bfc = nc.declare_dram_parameter("bfc", [1, OUT], F32R, isOutput=False)
    eye = nc.declare_dram_parameter("eye", [32, 32], F32, isOutput=False)
    zeros = nc.declare_dram_parameter("zeros", [128, NC, B], F32R, isOutput=False)
    ones = nc.declare_dram_parameter("ones", [1, B], F32R, isOutput=False)

    outp = nc.declare_dram_parameter("outp", [T, B, OUT], F32, isOutput=True)
    h_out = nc.declare_dram_parameter("h_out", [2, 128, NC, B], F32R, isOutput=True)
    c_out = nc.declare_dram_parameter("c_out", [2, B, 2, S], F32, isOutput=True)

    rg = [list(range(NC))]

    with tile.TileContext(nc) as tc:
        with (
            tc.tile_pool(name="wpool", bufs=1) as wpool,
            tc.tile_pool(name="xpool", bufs=3) as xpool,
            tc.tile_pool(name="hpool", bufs=2) as hpool,
            tc.tile_pool(name="epool", bufs=2) as epool,
            tc.tile_pool(name="spool", bufs=2) as spool,
            tc.tile_pool(name="psA", bufs=3, space="PSUM") as psA,
            tc.tile_pool(name="psB", bufs=2, space="PSUM") as psB,
            tc.tile_pool(name="psC", bufs=1, space="PSUM") as psC,
            tc.tile_pool(name="psT", bufs=1, space="PSUM") as psT,
            tc.tile_pool(name="dram", bufs=4, space="DRAM") as dram,
        ):
            # ---- static tiles ----
            w0x_sb = wpool.tile([128, 4, 2 * G], F32R, tag="w0x")
            w0h_sb = wpool.tile([128, NC, 2 * G], F32R, tag="w0h")
            wc_sb = wpool.tile([128, NC, 2 * G], F32R, tag="wc")
            w1x_sb = wpool.tile([128, NC, 2 * G], F32R, tag="w1x")
            w1h_sb = wpool.tile([128, NC, 2 * G], F32R, tag="w1h")
            wfc_sb = wpool.tile([128, NC, OUT], F32R, tag="wfc")
            b0_sb = wpool.tile([1, 2 * G], F32R, tag="b0")
            b0z_sb = wpool.tile([1, 2 * G], F32R, tag="b0z")
            b1_sb = wpool.tile([1, 2 * G], F32R, tag="b1")
            bfc_sb = wpool.tile([1, OUT], F32R, tag="bfc")
            eye_sb = wpool.tile([32, 32], F32, tag="eye")
            ones_sb = wpool.tile([1, B], F32R, tag="ones")

            for sb, dr in [(w0x_sb, w0x), (w0h_sb, w0h), (wc_sb, wcomb),
                           (w1x_sb, w1x), (w1h_sb, w1h), (wfc_sb, wfc)]:
                nc.sync.dma_start(sb[:], dr[:].rearrange("(k p) n -> p k n", p=128))
            for sb, dr in [(b0_sb, bias0), (b0z_sb, bias0z), (b1_sb, bias1), (bfc_sb, bfc),
                           (eye_sb, eye), (ones_sb, ones)]:
                nc.sync.dma_start(sb[:], dr[:])

            # ---- recurrent state (t = -1) ----
            h0T = hpool.tile([128, NC, B], F32R, tag="h0T")   # gathered h0.T
            h1T = hpool.tile([128, NC, B], F32R, tag="h1T")   # gathered h1.T
            tgc0 = epool.tile([B, 2, 2, S], F32, tag="tgc0")
            tgc1 = epool.tile([B, 2, 2, S], F32, tag="tgc1")
            nc.sync.dma_start(h0T[:], zeros[:])
            nc.sync.dma_start(h1T[:], zeros[:])
            nc.gpsimd.memset(tgc0[:], 0.0)
            nc.gpsimd.memset(tgc1[:], 0.0)

            ones_t = wpool.tile([B, 2, S], F32, tag="ones_t")
            nc.gpsimd.memset(ones_t[:], 1.0)

            def cell_elementwise(ps, tgc, tag):
                """LSTM cell pointwise math on gates in PSUM [B, 2, 4*S].

                Per-direction gate order is [i(S), f(S), o(S), g(S)]; the
                g-rows' weights are pre-scaled by 2 on the host, so one
                sigmoid covers all gates and tanh(g) = 2*sigmoid(2g) - 1.
                `tgc` is [B,2,2,S] with slot1 = c_prev (slot0 is scratch).
                Returns (h_new [B,2,S], tgc_next holding c_new in slot1)."""
                sg = epool.tile([B, 2, 4 * S], F32, tag=f"sig{tag}")
                nc.scalar.activation(sg[:], ps[:], AFT.Sigmoid)
                # tg = 2*sigmoid(2g) - 1 into scratch slot0 (pairs with c in slot1)
                nc.vector.scalar_tensor_tensor(
                    tgc[:, :, 0, :], sg[:, :, 3 * S:4 * S], 2.0, ones_t[:],
                    mybir.AluOpType.mult, mybir.AluOpType.subtract)
                prod = epool.tile([B, 2, 2, S], F32, tag=f"prod{tag}")
                nc.vector.tensor_mul(
                    prod[:], sg[:, :, 0:2 * S].rearrange("b d (u s) -> b d u s", u=2),
                    tgc[:])
                tgc_next = epool.tile([B, 2, 2, S], F32, tag=f"tgc{tag}")
                nc.vector.tensor_add(tgc_next[:, :, 1, :], prod[:, :, 0, :],
                                     prod[:, :, 1, :])
                tc_ = epool.tile([B, 2, S], F32, tag=f"tc{tag}")
                nc.scalar.activation(tc_[:], tgc_next[:, :, 1, :], AFT.Tanh)
                h_new = epool.tile([B, 2, S], F32, tag=f"h{tag}")
                nc.vector.tensor_mul(h_new[:], sg[:, :, 2 * S:3 * S], tc_[:])
                return h_new, tgc_next

            def sync(h_new, tag):
                """Transpose own h slice [B, 2*S] -> [128, B], AllGather,
                land gathered h.T j-major as [128(part), j, B]."""
                pst = psT.tile([128, B], F32, tag="pst")
                nc.tensor.transpose(
                    pst[:], h_new[:].rearrange("b d s -> b (d s)"), eye_sb[:]
                )
                stage = spool.tile([128, B], F32R, tag=f"stage{tag}")
                nc.vector.tensor_copy(stage[:], pst[:])
                in_b = dram.tile([128, B], F32R, tag="agin")
                out_b = dram.tile([NC * 128, B], F32R, tag="agout")
                nc.sync.dma_start(in_b[:], stage[:])
                if FAKE_AG:
                    ob = out_b[:].rearrange("(j r) b -> j r b", j=NC)
                    for j in range(NC):
                        nc.sync.dma_start(ob[j], in_b[:])
                else:
                    nc.gpsimd.collective_compute(
                        "AllGather", mybir.AluOpType.bypass, replica_groups=rg,
                        ins=[in_b[:].opt()], outs=[out_b[:].opt()],
                    )
                hT_new = hpool.tile([128, NC, B], F32R, tag=tag)
                nc.sync.dma_start(
                    hT_new[:], out_b[:].rearrange("(j p) b -> p j b", p=128)
                )
                return hT_new

            # L0 gates for step 0: bias + x + Whh/comb on zero-init h tiles
            psL0 = psA.tile([B, 2, G], F32, tag="psL0")
            nc.tensor.matmul(psL0[:], ones_sb[:], b0z_sb[:], start=True, stop=False)
            x_sb = xpool.tile([128, 4, 4, B], F32R, tag="x")
            nc.sync.dma_start(
                x_sb[:], xT[0:4].rearrange("t (k p) b -> p t k b", p=128))
            for m in range(4):
                nc.tensor.matmul(psL0[:], x_sb[:, 0, m, :], w0x_sb[:, m, :],
                                 start=False, stop=False)
            for j in range(NC):
                nc.tensor.matmul(psL0[:], h0T[:, j, :], w0h_sb[:, j, :],
                                 start=False, stop=False)
            for j in range(NC):
                nc.tensor.matmul(psL0[:], h1T[:, j, :], wc_sb[:, j, :],
                                 start=False, stop=(j == NC - 1))

            for t in range(T):
                # ---- L0 cell (gates fully accumulated) ----
                h0_new, tgc0 = cell_elementwise(psL0, tgc0, "0")

                # ---- sync #1: gather h0(t) ----
                h0T_new = sync(h0_new, "h0T")

                # overlaps AG#1: L1 Whh (h1(t-1)), next L0 bias+x
                psL1 = psB.tile([B, 2, G], F32, tag="psL1")
                nc.tensor.matmul(psL1[:], ones_sb[:], b1_sb[:], start=True, stop=False)
                for j in range(NC):
                    nc.tensor.matmul(psL1[:], h1T[:, j, :], w1h_sb[:, j, :],
                                     start=False, stop=False)
                if t + 1 < T:
                    psL0n = psA.tile([B, 2, G], F32, tag="psL0")
                    nc.tensor.matmul(psL0n[:], ones_sb[:], b0_sb[:],
                                     start=True, stop=False)
                    if (t + 1) % 4 == 0:
                        x_sbn = xpool.tile([128, 4, 4, B], F32R, tag="x")
                        hi = min(t + 5, T)
                        nc.sync.dma_start(
                            x_sbn[:, 0:hi - t - 1, :, :],
                            xT[t + 1:hi].rearrange("t (k p) b -> p t k b", p=128))
                        x_sb = x_sbn
                    for m in range(4):
                        nc.tensor.matmul(psL0n[:], x_sb[:, (t + 1) % 4, m, :],
                                         w0x_sb[:, m, :], start=False, stop=False)

                # ---- L1 Wih (needs gathered h0 of this step) ----
                h0T = h0T_new
                for j in range(NC):
                    nc.tensor.matmul(psL1[:], h0T[:, j, :], w1x_sb[:, j, :],
                                     start=False, stop=(j == NC - 1))

                # ---- L1 cell ----
                h1_new, tgc1 = cell_elementwise(psL1, tgc1, "1")

                # ---- sync #2: gather h1(t) ----
                h1T_new = sync(h1_new, "h1T")

                # overlaps AG#2: next L0 Whh (h0(t))
                if t + 1 < T:
                    for j in range(NC):
                        nc.tensor.matmul(psL0n[:], h0T[:, j, :], w0h_sb[:, j, :],
                                         start=False, stop=False)

                h1T = h1T_new
                # ---- critical-path tail: next L0 composed-pred part ----
                if t + 1 < T:
                    for j in range(NC):
                        nc.tensor.matmul(psL0n[:], h1T[:, j, :], wc_sb[:, j, :],
                                         start=False, stop=(j == NC - 1))

                # ---- FC head (output only; off the recurrence) ----
                psFC = psC.tile([B, OUT], F32, tag="psFC")
                nc.tensor.matmul(psFC[:], ones_sb[:], bfc_sb[:], start=True, stop=False)
                for j in range(NC):
                    nc.tensor.matmul(psFC[:], h1T[:, j, :], wfc_sb[:, j, :],
                                     start=False, stop=(j == NC - 1))
                if t % 4 == 0:
                    pred_sb = spool.tile([B, 4, OUT], F32, tag="pred")
                nc.vector.tensor_copy(pred_sb[:, t % 4, :], psFC[:])
                if t % 4 == 3 or t == T - 1:
                    lo = t - (t % 4)
                    nc.sync.dma_start(
                        outp[lo:t + 1].rearrange("t b o -> b t o"),
                        pred_sb[:, 0:t % 4 + 1, :])

                if t + 1 < T:
                    psL0 = psL0n

            # ---- final states ----
            nc.sync.dma_start(h_out[0], h0T[:])
            nc.sync.dma_start(h_out[1], h1T[:])
            nc.sync.dma_start(c_out[0], tgc0[:, :, 1, :])
            nc.sync.dma_start(c_out[1], tgc1[:, :, 1, :])

    nc.compile()
    return nc


# ----------------------------------------------------------------------------
# host-side sharding / unsharding
# ----------------------------------------------------------------------------

def _gate_rows(j):
    """Per-core gate row indices of one cell's [4*LH] gate dim, reordered to
    [i, f, o, g] (PyTorch layout is [i, f, g, o])."""
    base = j * S + np.arange(S)
    return np.concatenate([0 * LH + base, 1 * LH + base, 3 * LH + base, 2 * LH + base])


def _jmajor(h_major):
    """Permute an h-contraction [2*LH, N] matrix whose rows are ordered
    [dir0 units 0..511, dir1 units 0..511] into the gathered j-major row
    order (row j*128 + d*64 + i <-> dir d, unit j*64+i)."""
    r = np.arange(2 * LH)
    j, rem = divmod(r, 128)
    d, i = divmod(rem, S)
    idx = d * LH + j * S + i
    return h_major[idx]


def _scale_g(w):
    """Scale the g-gate columns (last S of each direction block) by 2."""
    w = w.copy()
    for d in range(2):
        w[..., d * G + 3 * S:d * G + 4 * S] *= 2
    return w


def make_in_maps(input_seq, W_ih, W_hh, b_ih, b_hh, W_fc, b_fc, T=T_FULL):
    input_seq = np.asarray(input_seq, np.float32)
    W_ih, W_hh = np.asarray(W_ih, np.float32), np.asarray(W_hh, np.float32)
    b_ih, b_hh = np.asarray(b_ih, np.float32), np.asarray(b_hh, np.float32)
    W_fc, b_fc = np.asarray(W_fc, np.float32), np.asarray(b_fc, np.float32)

    CDT = np.float16
    xT = np.ascontiguousarray(input_seq[:, :T, :].transpose(1, 2, 0)).astype(CDT)
    eye = np.eye(32, dtype=np.float32)
    ones = np.ones((1, B), CDT)
    wfc = np.ascontiguousarray(_jmajor(W_fc.T)).astype(CDT)  # [1024, 512], j-major
    W_fc64 = W_fc.astype(np.float64)
    b_fc64 = b_fc.astype(np.float64)

    in_maps = []
    for j in range(NC):
        rows = _gate_rows(j)
        w0x = np.concatenate([W_ih[0, d][rows][:, :IN].T for d in range(2)], axis=1)

        def hh(l):
            # per-direction block-diagonal Whh, rows h-major then j-major
            w = np.zeros((2 * LH, 2 * G), np.float32)
            for d in range(2):
                w[d * LH:(d + 1) * LH, d * G:(d + 1) * G] = W_hh[l, d][rows].T
            return np.ascontiguousarray(_jmajor(w))

        # composed pred feedback: (W0p_slice @ W_fc) over h1
        wc = np.concatenate(
            [(W_ih[0, d][rows][:, IN:].astype(np.float64) @ W_fc64).T.astype(np.float32)
             for d in range(2)], axis=1)
        w1x = np.concatenate([W_ih[1, d][rows].T for d in range(2)], axis=1)
        bias0 = np.concatenate(
            [(b_ih[0, d] + b_hh[0, d])[rows]
             + (W_ih[0, d][rows][:, IN:].astype(np.float64) @ b_fc64).astype(np.float32)
             for d in range(2)])
        bias0z = np.concatenate(
            [(b_ih[0, d] + b_hh[0, d])[rows] for d in range(2)])
        bias1 = np.concatenate([(b_ih[1, d] + b_hh[1, d])[rows] for d in range(2)])
        in_maps.append({
            "xT": xT,
            "w0x": _scale_g(np.ascontiguousarray(w0x)).astype(CDT),
            "w0h": _scale_g(hh(0)).astype(CDT),
            "wcomb": _scale_g(np.ascontiguousarray(_jmajor(wc))).astype(CDT),
            "w1x": _scale_g(np.ascontiguousarray(_jmajor(w1x))).astype(CDT),
            "w1h": _scale_g(hh(1)).astype(CDT),
            "wfc": wfc,
            "bias0": _scale_g(bias0.reshape(1, -1)).astype(CDT),
            "bias0z": _scale_g(bias0z.reshape(1, -1)).astype(CDT),
            "bias1": _scale_g(bias1.reshape(1, -1)).astype(CDT),
            "bfc": b_fc.reshape(1, OUT).astype(CDT), "eye": eye, "ones": ones,
            "zeros": np.zeros((128, NC, B), CDT),
        })
    return in_maps


def unshard(results, T=T_FULL):
    outputs = np.ascontiguousarray(results[0]["outp"].transpose(1, 0, 2))  # [B,T,OUT]
    hT = results[0]["h_out"]  # [2, 128(p = d*64+i), NC(j), B]
    h_n = np.empty((4, B, LH), np.float32)
    for l in range(2):
        for d in range(2):
            blk = hT[l, d * S:(d + 1) * S, :, :]          # [i, j, b]
            h_n[2 * l + d] = blk.transpose(1, 0, 2).reshape(LH, B).T
    c_n = np.empty((4, B, LH), np.float32)
    for j in range(NC):
        cj = results[j]["c_out"]  # [2, B, 2, S]
        for l in range(2):
            for d in range(2):
                c_n[2 * l + d][:, j * S:(j + 1) * S] = cj[l, :, d, :]
    return outputs, h_n, c_n


_CACHE = {}


def kernel(input_seq, input_lengths, W_ih, W_hh, b_ih, b_hh, W_fc, b_fc):
    T = input_seq.shape[1]
    if T not in _CACHE:
        _CACHE[T] = build(T)
    nc = _CACHE[T]
    in_maps = make_in_maps(input_seq, W_ih, W_hh, b_ih, b_hh, W_fc, b_fc, T=T)
    res = run_bass_kernel_spmd(nc, in_maps, core_ids=list(range(NC)))
    return unshard(res.results, T=T)


# revision 19
# speedup vs baseline: 1.7743x; 1.7743x over previous
"""Trainium2 Bass kernel for nn_AutoregressiveEncoder (8 NeuronCores).

Model (see reference): per timestep t, a 2-layer bidirectional LSTM applied to
a length-1 sequence [x_t ; pred_{t-1}], hidden state carried across t, then a
linear head; the prediction feeds back into the next step's input.

Strategy: 8-way tensor parallel, gate-dim sliced (each core owns 64 h-units
per direction per layer = 256 of the 2048 gate rows per cell); batch (32) kept
whole. Per step, two AllGathers distribute the per-core h-slices. The pred
feedback is algebraically composed away: W0p @ pred = (W0p @ W_fc) @ h1 +
W0p @ b_fc, so the FC head (replicated, fed by gathered h1) only produces the
output tensor and is off the serial critical path.

Matmul layout ("weights moving"): activations live feature-major and serve as
the 128x32 stationary operand; weights stream as the [128, <=512] moving
operand in float32r (TF32-like, 1 cyc/row). Gates land batch-major [32, 512]
in PSUM where the LSTM cell elementwise runs. Gathered h tensors are kept in
the AllGather's natural j-major block order; weight rows are host-permuted to
match, and per-direction Whh blocks are zero-padded block-diagonal so one
N=512 matmul per gathered K-tile covers both directions.

Hardcoded shapes: B=32, T=512, IN=OUT=512, LH=512 (per-direction), L=2.
"""
import os
import numpy as np

import concourse.bacc as bacc
import concourse.mybir as mybir
import concourse.tile as tile
from concourse.bass_utils import run_bass_kernel_spmd

B, T_FULL, IN, OUT = 32, 512, 512, 512
LH = 512          # per-direction hidden size
NC = 8            # cores
S = LH // NC      # h-units per direction per core (64)
G = 4 * S         # gate rows per direction per core (256)

F32 = mybir.dt.float32
F32R = mybir.dt.float16  # compute dtype for PE-facing tensors
AFT = mybir.ActivationFunctionType

FAKE_AG = bool(int(os.environ.get("FAKE_AG", "0")))


def build(T=T_FULL):
    nc = bacc.Bacc(trn_type="TRN2", num_devices=NC)

    # ---- DRAM parameters (per core) ----
    # weight row order for h-contractions is the gathered j-major order:
    # row (j*128 + d*64 + i) <-> direction d, unit j*64+i
    xT = nc.declare_dram_parameter("xT", [T, IN, B], F32R, isOutput=False)
    w0x = nc.declare_dram_parameter("w0x", [IN, 2 * G], F32R, isOutput=False)
    w0h = nc.declare_dram_parameter("w0h", [2 * LH, 2 * G], F32R, isOutput=False)
    wcomb = nc.declare_dram_parameter("wcomb", [2 * LH, 2 * G], F32R, isOutput=False)
    w1x = nc.declare_dram_parameter("w1x", [2 * LH, 2 * G], F32R, isOutput=False)
    w1h = nc.declare_dram_parameter("w1h", [2 * LH, 2 * G], F32R, isOutput=False)
    wfc = nc.declare_dram_parameter("wfc", [2 * LH, OUT], F32R, isOutput=False)
    bias0 = nc.declare_dram_parameter("bias0", [1, 2 * G], F32R, isOutput=False)
    bias0z = nc.declare_dram_parameter("bias0z", [1, 2 * G], F32R, isOutput=False)
    bias1 = nc.declare_dram_parameter("bias1", [1, 2 * G], F32R, isOutput=False)
    bfc = nc.declare_dram_parameter("bfc", [1, OUT], F32R, isOutput=False)
    eye = nc.declare_dram_parameter("eye", [32, 32], F32, isOutput=False)
    zeros = nc.declare_dram_parameter("zeros", [128, NC, B], F32R, isOutput=False)
    ones = nc.declare_dram_parameter("ones", [1, B], F32R, isOutput=False)

    outp = nc.declare_dram_parameter("outp", [T, B, OUT], F32R, isOutput=True)
    h_out = nc.declare_dram_parameter("h_out", [2, 128, NC, B], F32R, isOutput=True)
    c_out = nc.declare_dram_parameter("c_out", [2, B, 2, S], F32, isOutput=True)

    rg = [list(range(NC))]

    with tile.TileContext(nc) as tc:
        with (
            tc.tile_pool(name="wpool", bufs=1) as wpool,
            tc.tile_pool(name="xpool", bufs=3) as xpool,
            tc.tile_pool(name="hpool", bufs=3) as hpool,
            tc.tile_pool(name="epool", bufs=3) as epool,
            tc.tile_pool(name="spool", bufs=3) as spool,
            tc.tile_pool(name="psA", bufs=3, space="PSUM") as psA,
            tc.tile_pool(name="psB", bufs=2, space="PSUM") as psB,
            tc.tile_pool(name="psC", bufs=1, space="PSUM") as psC,
            tc.tile_pool(name="dram", bufs=4, space="DRAM") as dram,
        ):
            # ---- static tiles ----
            w0x_sb = wpool.tile([128, 4, 2 * G], F32R, tag="w0x")
            w0h_sb = wpool.tile([128, NC, 2 * G], F32R, tag="w0h")
            wc_sb = wpool.tile([128, NC, 2 * G], F32R, tag="wc")
            w1x_sb = wpool.tile([128, NC, 2 * G], F32R, tag="w1x")
            w1h_sb = wpool.tile([128, NC, 2 * G], F32R, tag="w1h")
            wfc_sb = wpool.tile([128, NC, OUT], F32R, tag="wfc")
            b0_sb = wpool.tile([1, 2 * G], F32R, tag="b0")
            b0z_sb = wpool.tile([1, 2 * G], F32R, tag="b0z")
            b1_sb = wpool.tile([1, 2 * G], F32R, tag="b1")
            bfc_sb = wpool.tile([1, OUT], F32R, tag="bfc")
            eye_sb = wpool.tile([32, 32], F32, tag="eye")
            ones_sb = wpool.tile([1, B], F32R, tag="ones")

            for sb, dr in [(w0x_sb, w0x), (w0h_sb, w0h), (wc_sb, wcomb),
                           (w1x_sb, w1x), (w1h_sb, w1h), (wfc_sb, wfc)]:
                nc.sync.dma_start(sb[:], dr[:].rearrange("(k p) n -> p k n", p=128))
            for sb, dr in [(b0_sb, bias0), (b0z_sb, bias0z), (b1_sb, bias1), (bfc_sb, bfc),
                           (eye_sb, eye), (ones_sb, ones)]:
                nc.sync.dma_start(sb[:], dr[:])

            # ---- recurrent state (t = -1) ----
            h0T = hpool.tile([128, NC, B], F32R, tag="h0T")   # gathered h0.T
            h1T = hpool.tile([128, NC, B], F32R, tag="h1T")   # gathered h1.T
            tgc0 = epool.tile([B, 2, 2, S], F32, tag="tgc0")
            tgc1 = epool.tile([B, 2, 2, S], F32, tag="tgc1")
            nc.sync.dma_start(h0T[:], zeros[:])
            nc.sync.dma_start(h1T[:], zeros[:])
            nc.gpsimd.memset(tgc0[:], 0.0)
            nc.gpsimd.memset(tgc1[:], 0.0)

            ones_t = wpool.tile([B, 2, S], F32, tag="ones_t")
            nc.gpsimd.memset(ones_t[:], 1.0)

            def cell_elementwise(ps, tgc, tag):
                """LSTM cell pointwise math on gates in PSUM [B, 2, 4*S].

                Per-direction gate order is [i(S), f(S), o(S), g(S)]; the
                g-rows' weights are pre-scaled by 2 on the host, so one
                sigmoid covers all gates and tanh(g) = 2*sigmoid(2g) - 1.
                `tgc` is [B,2,2,S] with slot1 = c_prev (slot0 is scratch).
                Returns (h_new [B,2,S], tgc_next holding c_new in slot1)."""
                sg = epool.tile([B, 2, 4 * S], F32, tag=f"sig{tag}")
                nc.scalar.activation(sg[:], ps[:], AFT.Sigmoid)
                # tg = 2*sigmoid(2g) - 1 into scratch slot0 (pairs with c in slot1)
                nc.vector.scalar_tensor_tensor(
                    tgc[:, :, 0, :], sg[:, :, 3 * S:4 * S], 2.0, ones_t[:],
                    mybir.AluOpType.mult, mybir.AluOpType.subtract)
                prod = epool.tile([B, 2, 2, S], F32, tag=f"prod{tag}")
                nc.vector.tensor_mul(
                    prod[:], sg[:, :, 0:2 * S].rearrange("b d (u s) -> b d u s", u=2),
                    tgc[:])
                tgc_next = epool.tile([B, 2, 2, S], F32, tag=f"tgc{tag}")
                nc.vector.tensor_add(tgc_next[:, :, 1, :], prod[:, :, 0, :],
                                     prod[:, :, 1, :])
                tc_ = epool.tile([B, 2, S], F32, tag=f"tc{tag}")
                nc.scalar.activation(tc_[:], tgc_next[:, :, 1, :], AFT.Tanh)
                h_new = epool.tile([B, 2, S], F32R, tag=f"h{tag}")
                nc.vector.tensor_mul(h_new[:], sg[:, :, 2 * S:3 * S], tc_[:])
                return h_new, tgc_next

            def sync(h_new, tag):
                """DMA-transpose own h slice [B, 2*S] -> [128, B], AllGather,
                land gathered h.T j-major as [128(part), j, B]."""
                stage = spool.tile([128, B], F32R, tag=f"stage{tag}")
                nc.sync.dma_start_transpose(
                    out=stage[:], in_=h_new[:].rearrange("b d s -> b (d s)"))
                in_b = dram.tile([128, B], F32R, tag="agin")
                out_b = dram.tile([NC * 128, B], F32R, tag="agout")
                nc.gpsimd.dma_start(in_b[:], stage[:])
                if FAKE_AG:
                    ob = out_b[:].rearrange("(j r) b -> j r b", j=NC)
                    for j in range(NC):
                        nc.sync.dma_start(ob[j], in_b[:])
                else:
                    nc.gpsimd.collective_compute(
                        "AllGather", mybir.AluOpType.bypass, replica_groups=rg,
                        ins=[in_b[:].opt()], outs=[out_b[:].opt()],
                    )
                hT_new = hpool.tile([128, NC, B], F32R, tag=tag)
                nc.gpsimd.dma_start(
                    hT_new[:], out_b[:].rearrange("(j p) b -> p j b", p=128)
                )
                return hT_new

            # L0 gates for step 0: bias + x + Whh/comb on zero-init h tiles
            psL0 = psA.tile([B, 2, G], F32, tag="psL0")
            nc.tensor.matmul(psL0[:], ones_sb[:], b0z_sb[:], start=True, stop=False)
            x_sb = xpool.tile([128, 4, 4, B], F32R, tag="x")
            nc.sync.dma_start(
                x_sb[:], xT[0:4].rearrange("t (k p) b -> p t k b", p=128))
            for m in range(4):
                nc.tensor.matmul(psL0[:], x_sb[:, 0, m, :], w0x_sb[:, m, :],
                                 start=False, stop=False)
            for j in range(NC):
                nc.tensor.matmul(psL0[:], h0T[:, j, :], w0h_sb[:, j, :],
                                 start=False, stop=False)
            for j in range(NC):
                nc.tensor.matmul(psL0[:], h1T[:, j, :], wc_sb[:, j, :],
                                 start=False, stop=(j == NC - 1))

            for t in range(T):
                # ---- L0 cell (gates fully accumulated) ----
                h0_new, tgc0 = cell_elementwise(psL0, tgc0, "0")

                # ---- sync #1: gather h0(t) ----
                h0T_new = sync(h0_new, "h0T")

                # overlaps AG#1: L1 Whh (h1(t-1)), next L0 bias+x
                psL1 = psB.tile([B, 2, G], F32, tag="psL1")
                nc.tensor.matmul(psL1[:], ones_sb[:], b1_sb[:], start=True, stop=False)
                for j in range(NC):
                    nc.tensor.matmul(psL1[:], h1T[:, j, :], w1h_sb[:, j, :],
                                     start=False, stop=False)
                if t + 1 < T:
                    psL0n = psA.tile([B, 2, G], F32, tag="psL0")
                    nc.tensor.matmul(psL0n[:], ones_sb[:], b0_sb[:],
                                     start=True, stop=False)
                    if (t + 1) % 4 == 0:
                        x_sbn = xpool.tile([128, 4, 4, B], F32R, tag="x")
                        hi = min(t + 5, T)
                        nc.sync.dma_start(
                            x_sbn[:, 0:hi - t - 1, :, :],
                            xT[t + 1:hi].rearrange("t (k p) b -> p t k b", p=128))
                        x_sb = x_sbn
                    for m in range(4):
                        nc.tensor.matmul(psL0n[:], x_sb[:, (t + 1) % 4, m, :],
                                         w0x_sb[:, m, :], start=False, stop=False)

                # ---- L1 Wih (needs gathered h0 of this step) ----
                h0T = h0T_new
                for j in range(NC):
                    nc.tensor.matmul(psL1[:], h0T[:, j, :], w1x_sb[:, j, :],
                                     start=False, stop=(j == NC - 1))

                # ---- L1 cell ----
                h1_new, tgc1 = cell_elementwise(psL1, tgc1, "1")

                # ---- sync #2: gather h1(t) ----
                h1T_new = sync(h1_new, "h1T")

                # overlaps AG#2: next L0 Whh (h0(t))
                if t + 1 < T:
                    for j in range(NC):
                        nc.tensor.matmul(psL0n[:], h0T[:, j, :], w0h_sb[:, j, :],
                                         start=False, stop=False)

                h1T = h1T_new
                # ---- critical-path tail: next L0 composed-pred part ----
                if t + 1 < T:
                    for j in range(NC):
                        nc.tensor.matmul(psL0n[:], h1T[:, j, :], wc_sb[:, j, :],
                                         start=False, stop=(j == NC - 1))

                # ---- FC head (output only; off the recurrence) ----
                psFC = psC.tile([B, OUT], F32, tag="psFC")
                nc.tensor.matmul(psFC[:], ones_sb[:], bfc_sb[:], start=True, stop=False)
                for j in range(NC):
                    nc.tensor.matmul(psFC[:], h1T[:, j, :], wfc_sb[:, j, :],
                                     start=False, stop=(j == NC - 1))
                if t % 4 == 0:
                    pred_sb = spool.tile([B, 4, OUT], F32R, tag="pred")
                nc.vector.tensor_copy(pred_sb[:, t % 4, :], psFC[:])
                if t % 4 == 3 or t == T - 1:
                    lo = t - (t % 4)
                    nc.sync.dma_start(
                        outp[lo:t + 1].rearrange("t b o -> b t o"),
                        pred_sb[:, 0:t % 4 + 1, :])

                if t + 1 < T:
                    psL0 = psL0n

            # ---- final states ----
            nc.sync.dma_start(h_out[0], h0T[:])
            nc.sync.dma_start(h_out[1], h1T[:])
            nc.sync.dma_start(c_out[0], tgc0[:, :, 1, :])
            nc.sync.dma_start(c_out[1], tgc1[:, :, 1, :])

    nc.compile()
    return nc


# ----------------------------------------------------------------------------
# host-side sharding / unsharding
# ----------------------------------------------------------------------------

def _gate_rows(j):
    """Per-core gate row indices of one cell's [4*LH] gate dim, reordered to
    [i, f, o, g] (PyTorch layout is [i, f, g, o])."""
    base = j * S + np.arange(S)
    return np.concatenate([0 * LH + base, 1 * LH + base, 3 * LH + base, 2 * LH + base])


def _jmajor(h_major):
    """Permute an h-contraction [2*LH, N] matrix whose rows are ordered
    [dir0 units 0..511, dir1 units 0..511] into the gathered j-major row
    order (row j*128 + d*64 + i <-> dir d, unit j*64+i)."""
    r = np.arange(2 * LH)
    j, rem = divmod(r, 128)
    d, i = divmod(rem, S)
    idx = d * LH + j * S + i
    return h_major[idx]


def _scale_g(w):
    """Scale the g-gate columns (last S of each direction block) by 2."""
    w = w.copy()
    for d in range(2):
        w[..., d * G + 3 * S:d * G + 4 * S] *= 2
    return w


def make_in_maps(input_seq, W_ih, W_hh, b_ih, b_hh, W_fc, b_fc, T=T_FULL):
    input_seq = np.asarray(input_seq, np.float32)
    W_ih, W_hh = np.asarray(W_ih, np.float32), np.asarray(W_hh, np.float32)
    b_ih, b_hh = np.asarray(b_ih, np.float32), np.asarray(b_hh, np.float32)
    W_fc, b_fc = np.asarray(W_fc, np.float32), np.asarray(b_fc, np.float32)

    CDT = np.float16
    xT = np.ascontiguousarray(input_seq[:, :T, :].transpose(1, 2, 0)).astype(CDT)
    eye = np.eye(32, dtype=np.float32)
    ones = np.ones((1, B), CDT)
    wfc = np.ascontiguousarray(_jmajor(W_fc.T)).astype(CDT)  # [1024, 512], j-major
    W_fc64 = W_fc.astype(np.float64)
    b_fc64 = b_fc.astype(np.float64)

    in_maps = []
    for j in range(NC):
        rows = _gate_rows(j)
        w0x = np.concatenate([W_ih[0, d][rows][:, :IN].T for d in range(2)], axis=1)

        def hh(l):
            # per-direction block-diagonal Whh, rows h-major then j-major
            w = np.zeros((2 * LH, 2 * G), np.float32)
            for d in range(2):
                w[d * LH:(d + 1) * LH, d * G:(d + 1) * G] = W_hh[l, d][rows].T
            return np.ascontiguousarray(_jmajor(w))

        # composed pred feedback: (W0p_slice @ W_fc) over h1
        wc = np.concatenate(
            [(W_ih[0, d][rows][:, IN:].astype(np.float64) @ W_fc64).T.astype(np.float32)
             for d in range(2)], axis=1)
        w1x = np.concatenate([W_ih[1, d][rows].T for d in range(2)], axis=1)
        bias0 = np.concatenate(
            [(b_ih[0, d] + b_hh[0, d])[rows]
             + (W_ih[0, d][rows][:, IN:].astype(np.float64) @ b_fc64).astype(np.float32)
             for d in range(2)])
        bias0z = np.concatenate(
            [(b_ih[0, d] + b_hh[0, d])[rows] for d in range(2)])
        bias1 = np.concatenate([(b_ih[1, d] + b_hh[1, d])[rows] for d in range(2)])
        in_maps.append({
            "xT": xT,
            "w0x": _scale_g(np.ascontiguousarray(w0x)).astype(CDT),
            "w0h": _scale_g(hh(0)).astype(CDT),
            "wcomb": _scale_g(np.ascontiguousarray(_jmajor(wc))).astype(CDT),
            "w1x": _scale_g(np.ascontiguousarray(_jmajor(w1x))).astype(CDT),
            "w1h": _scale_g(hh(1)).astype(CDT),
            "wfc": wfc,
            "bias0": _scale_g(bias0.reshape(1, -1)).astype(CDT),
            "bias0z": _scale_g(bias0z.reshape(1, -1)).astype(CDT),
            "bias1": _scale_g(bias1.reshape(1, -1)).astype(CDT),
            "bfc": b_fc.reshape(1, OUT).astype(CDT), "eye": eye, "ones": ones,
            "zeros": np.zeros((128, NC, B), CDT),
        })
    return in_maps


def unshard(results, T=T_FULL):
    outputs = results[0]["outp"].transpose(1, 0, 2).astype(np.float32)  # [B,T,OUT]
    hT = results[0]["h_out"]  # [2, 128(p = d*64+i), NC(j), B]
    h_n = np.empty((4, B, LH), np.float32)
    for l in range(2):
        for d in range(2):
            blk = hT[l, d * S:(d + 1) * S, :, :]          # [i, j, b]
            h_n[2 * l + d] = blk.transpose(1, 0, 2).reshape(LH, B).T
    c_n = np.empty((4, B, LH), np.float32)
    for j in range(NC):
        cj = results[j]["c_out"]  # [2, B, 2, S]
        for l in range(2):
            for d in range(2):
                c_n[2 * l + d][:, j * S:(j + 1) * S] = cj[l, :, d, :]
    return outputs, h_n, c_n


_CACHE = {}


def kernel(input_seq, input_lengths, W_ih, W_hh, b_ih, b_hh, W_fc, b_fc):
    T = input_seq.shape[1]
    if T not in _CACHE:
        _CACHE[T] = build(T)
    nc = _CACHE[T]
    in_maps = make_in_maps(input_seq, W_ih, W_hh, b_ih, b_hh, W_fc, b_fc, T=T)
    res = run_bass_kernel_spmd(nc, in_maps, core_ids=list(range(NC)))
    return unshard(res.results, T=T)


# revision 20
# speedup vs baseline: 2.2618x; 1.2747x over previous
"""Trainium2 Bass kernel for nn_AutoregressiveEncoder (8 NeuronCores).

Model (see reference): per timestep t, a 2-layer bidirectional LSTM applied to
a length-1 sequence [x_t ; pred_{t-1}], hidden state carried across t, then a
linear head; the prediction feeds back into the next step's input.

Strategy: 8-way tensor parallel, gate-dim sliced (each core owns 64 h-units
per direction per layer = 256 of the 2048 gate rows per cell); batch (32) kept
whole. Per step, two AllGathers distribute the per-core h-slices. The pred
feedback is algebraically composed away: W0p @ pred = (W0p @ W_fc) @ h1 +
W0p @ b_fc, so the FC head (replicated, fed by gathered h1) only produces the
output tensor and is off the serial critical path.

Matmul layout ("weights moving"): activations live feature-major and serve as
the 128x32 stationary operand; weights stream as the [128, <=512] moving
operand in float32r (TF32-like, 1 cyc/row). Gates land batch-major [32, 512]
in PSUM where the LSTM cell elementwise runs. Gathered h tensors are kept in
the AllGather's natural j-major block order; weight rows are host-permuted to
match, and per-direction Whh blocks are zero-padded block-diagonal so one
N=512 matmul per gathered K-tile covers both directions.

Hardcoded shapes: B=32, T=512, IN=OUT=512, LH=512 (per-direction), L=2.
"""
import os
import numpy as np

import concourse.bacc as bacc
import concourse.mybir as mybir
import concourse.tile as tile
from concourse.bass_utils import run_bass_kernel_spmd

B, T_FULL, IN, OUT = 32, 512, 512, 512
LH = 512          # per-direction hidden size
NC = 8            # cores
S = LH // NC      # h-units per direction per core (64)
G = 4 * S         # gate rows per direction per core (256)

F32 = mybir.dt.float32
F32R = mybir.dt.float16  # compute dtype for PE-facing tensors
AFT = mybir.ActivationFunctionType

FAKE_AG = bool(int(os.environ.get("FAKE_AG", "0")))


def build(T=T_FULL):
    nc = bacc.Bacc(trn_type="TRN2", num_devices=NC)

    # ---- DRAM parameters (per core) ----
    # weight row order for h-contractions is the gathered j-major order:
    # row (j*128 + d*64 + i) <-> direction d, unit j*64+i
    xT = nc.declare_dram_parameter("xT", [T, IN, B], F32R, isOutput=False)
    w0x = nc.declare_dram_parameter("w0x", [IN, 2 * G], F32R, isOutput=False)
    w0h = nc.declare_dram_parameter("w0h", [2 * LH, 2 * G], F32R, isOutput=False)
    wcomb = nc.declare_dram_parameter("wcomb", [2 * LH, 2 * G], F32R, isOutput=False)
    w1x = nc.declare_dram_parameter("w1x", [2 * LH, 2 * G], F32R, isOutput=False)
    w1h = nc.declare_dram_parameter("w1h", [2 * LH, 2 * G], F32R, isOutput=False)
    wfc = nc.declare_dram_parameter("wfc", [2 * LH, OUT], F32R, isOutput=False)
    bias0 = nc.declare_dram_parameter("bias0", [1, 2 * G], F32R, isOutput=False)
    bias0z = nc.declare_dram_parameter("bias0z", [1, 2 * G], F32R, isOutput=False)
    bias1 = nc.declare_dram_parameter("bias1", [1, 2 * G], F32R, isOutput=False)
    bfc = nc.declare_dram_parameter("bfc", [1, OUT], F32R, isOutput=False)
    eye = nc.declare_dram_parameter("eye", [32, 32], F32, isOutput=False)
    zeros = nc.declare_dram_parameter("zeros", [128, NC, B], F32R, isOutput=False)
    ones = nc.declare_dram_parameter("ones", [1, B], F32R, isOutput=False)

    outp = nc.declare_dram_parameter("outp", [T, B, OUT], F32R, isOutput=True)
    h_out = nc.declare_dram_parameter("h_out", [2, 128, NC, B], F32R, isOutput=True)
    c_out = nc.declare_dram_parameter("c_out", [2, B, 2, S], F32, isOutput=True)

    rg = [list(range(NC))]

    with tile.TileContext(nc) as tc:
        with (
            tc.tile_pool(name="wpool", bufs=1) as wpool,
            tc.tile_pool(name="xpool", bufs=3) as xpool,
            tc.tile_pool(name="hpool", bufs=3) as hpool,
            tc.tile_pool(name="epool", bufs=3) as epool,
            tc.tile_pool(name="spool", bufs=3) as spool,
            tc.tile_pool(name="psA", bufs=3, space="PSUM") as psA,
            tc.tile_pool(name="psB", bufs=2, space="PSUM") as psB,
            tc.tile_pool(name="psC", bufs=1, space="PSUM") as psC,
            tc.tile_pool(name="dram", bufs=4, space="DRAM") as dram,
        ):
            # ---- static tiles ----
            w0x_sb = wpool.tile([128, 4, 2 * G], F32R, tag="w0x")
            w0h_sb = wpool.tile([128, NC, 2 * G], F32R, tag="w0h")
            wc_sb = wpool.tile([128, NC, 2 * G], F32R, tag="wc")
            w1x_sb = wpool.tile([128, NC, 2 * G], F32R, tag="w1x")
            w1h_sb = wpool.tile([128, NC, 2 * G], F32R, tag="w1h")
            wfc_sb = wpool.tile([128, NC, OUT], F32R, tag="wfc")
            b0_sb = wpool.tile([1, 2 * G], F32R, tag="b0")
            b0z_sb = wpool.tile([1, 2 * G], F32R, tag="b0z")
            b1_sb = wpool.tile([1, 2 * G], F32R, tag="b1")
            bfc_sb = wpool.tile([1, OUT], F32R, tag="bfc")
            eye_sb = wpool.tile([32, 32], F32, tag="eye")
            ones_sb = wpool.tile([1, B], F32R, tag="ones")

            for sb, dr in [(w0x_sb, w0x), (w0h_sb, w0h), (wc_sb, wcomb),
                           (w1x_sb, w1x), (w1h_sb, w1h), (wfc_sb, wfc)]:
                nc.sync.dma_start(sb[:], dr[:].rearrange("(k p) n -> p k n", p=128))
            for sb, dr in [(b0_sb, bias0), (b0z_sb, bias0z), (b1_sb, bias1), (bfc_sb, bfc),
                           (eye_sb, eye), (ones_sb, ones)]:
                nc.sync.dma_start(sb[:], dr[:])

            # ---- recurrent state (t = -1) ----
            h0T = hpool.tile([128, NC, B], F32R, tag="h0T")   # gathered h0.T
            h1T = hpool.tile([128, NC, B], F32R, tag="h1T")   # gathered h1.T
            tgc0 = epool.tile([B, 2, 2, S], F32, tag="tgc0")
            tgc1 = epool.tile([B, 2, 2, S], F32, tag="tgc1")
            nc.sync.dma_start(h0T[:], zeros[:])
            nc.sync.dma_start(h1T[:], zeros[:])
            nc.gpsimd.memset(tgc0[:], 0.0)
            nc.gpsimd.memset(tgc1[:], 0.0)

            ones_t = wpool.tile([B, 2, S], F32, tag="ones_t")
            nc.gpsimd.memset(ones_t[:], 1.0)

            def cell_elementwise(ps, tgc, tag):
                """LSTM cell pointwise math on gates in PSUM [B, 2, 4*S].

                Per-direction gate order is [i(S), f(S), o(S), g(S)]; the
                g-rows' weights are pre-scaled by 2 on the host, so one
                sigmoid covers all gates and tanh(g) = 2*sigmoid(2g) - 1.
                `tgc` is [B,2,2,S] with slot1 = c_prev (slot0 is scratch).
                Returns (h_new [B,2,S], tgc_next holding c_new in slot1)."""
                sg = epool.tile([B, 2, 4 * S], F32, tag=f"sig{tag}")
                nc.scalar.activation(sg[:], ps[:], AFT.Sigmoid)
                # tg = 2*sigmoid(2g) - 1 into scratch slot0 (pairs with c in slot1)
                nc.vector.scalar_tensor_tensor(
                    tgc[:, :, 0, :], sg[:, :, 3 * S:4 * S], 2.0, ones_t[:],
                    mybir.AluOpType.mult, mybir.AluOpType.subtract)
                prod = epool.tile([B, 2, 2, S], F32, tag=f"prod{tag}")
                nc.vector.tensor_mul(
                    prod[:], sg[:, :, 0:2 * S].rearrange("b d (u s) -> b d u s", u=2),
                    tgc[:])
                tgc_next = epool.tile([B, 2, 2, S], F32, tag=f"tgc{tag}")
                nc.vector.tensor_add(tgc_next[:, :, 1, :], prod[:, :, 0, :],
                                     prod[:, :, 1, :])
                tc_ = epool.tile([B, 2, S], F32, tag=f"tc{tag}")
                nc.scalar.activation(tc_[:], tgc_next[:, :, 1, :], AFT.Tanh)
                h_new = epool.tile([B, 2, S], F32R, tag=f"h{tag}")
                nc.vector.tensor_mul(h_new[:], sg[:, :, 2 * S:3 * S], tc_[:])
                return h_new, tgc_next

            def sync(h_new, tag):
                """DMA-transpose own h slice [B, 2*S] -> [128, B], AllGather,
                land gathered h.T j-major as [128(part), j, B]."""
                stage = spool.tile([128, B], F32R, tag=f"stage{tag}")
                nc.sync.dma_start_transpose(
                    out=stage[:], in_=h_new[:].rearrange("b d s -> b (d s)"))
                in_b = dram.tile([128, B], F32R, tag="agin")
                out_b = dram.tile([NC * 128, B], F32R, tag="agout")
                nc.gpsimd.dma_start(in_b[:], stage[:])
                if FAKE_AG == 2:
                    # timing ablation: zero-comm lower bound (wrong values)
                    nc.sync.dma_start(out_b[0:128, :], in_b[:])
                elif FAKE_AG:
                    ob = out_b[:].rearrange("(j r) b -> j r b", j=NC)
                    for j in range(NC):
                        nc.sync.dma_start(ob[j], in_b[:])
                else:
                    nc.gpsimd.collective_compute(
                        "AllGather", mybir.AluOpType.bypass, replica_groups=rg,
                        ins=[in_b[:].opt()], outs=[out_b[:].opt()],
                    )
                hT_new = hpool.tile([128, NC, B], F32R, tag=tag)
                nc.gpsimd.dma_start(
                    hT_new[:], out_b[:].rearrange("(j p) b -> p j b", p=128)
                )
                return hT_new

            # L0 gates for step 0: bias + x + Whh/comb on zero-init h tiles
            psL0 = psA.tile([B, 2, G], F32, tag="psL0")
            nc.tensor.matmul(psL0[:], ones_sb[:], b0z_sb[:], start=True, stop=False)
            x_sb = xpool.tile([128, 4, 4, B], F32R, tag="x")
            nc.sync.dma_start(
                x_sb[:], xT[0:4].rearrange("t (k p) b -> p t k b", p=128))
            for m in range(4):
                nc.tensor.matmul(psL0[:], x_sb[:, 0, m, :], w0x_sb[:, m, :],
                                 start=False, stop=False)
            for j in range(NC):
                nc.tensor.matmul(psL0[:], h0T[:, j, :], w0h_sb[:, j, :],
                                 start=False, stop=False)
            for j in range(NC):
                nc.tensor.matmul(psL0[:], h1T[:, j, :], wc_sb[:, j, :],
                                 start=False, stop=(j == NC - 1))

            for t in range(T):
                # ---- L0 cell (gates fully accumulated) ----
                h0_new, tgc0 = cell_elementwise(psL0, tgc0, "0")

                # ---- sync #1: gather h0(t) ----
                h0T_new = sync(h0_new, "h0T")

                # overlaps AG#1: L1 Whh (h1(t-1)), next L0 bias+x
                psL1 = psB.tile([B, 2, G], F32, tag="psL1")
                nc.tensor.matmul(psL1[:], ones_sb[:], b1_sb[:], start=True, stop=False)
                for j in range(NC):
                    nc.tensor.matmul(psL1[:], h1T[:, j, :], w1h_sb[:, j, :],
                                     start=False, stop=False)
                if t + 1 < T:
                    psL0n = psA.tile([B, 2, G], F32, tag="psL0")
                    nc.tensor.matmul(psL0n[:], ones_sb[:], b0_sb[:],
                                     start=True, stop=False)
                    if (t + 1) % 4 == 0:
                        x_sbn = xpool.tile([128, 4, 4, B], F32R, tag="x")
                        hi = min(t + 5, T)
                        nc.sync.dma_start(
                            x_sbn[:, 0:hi - t - 1, :, :],
                            xT[t + 1:hi].rearrange("t (k p) b -> p t k b", p=128))
                        x_sb = x_sbn
                    for m in range(4):
                        nc.tensor.matmul(psL0n[:], x_sb[:, (t + 1) % 4, m, :],
                                         w0x_sb[:, m, :], start=False, stop=False)

                # ---- L1 Wih (needs gathered h0 of this step) ----
                h0T = h0T_new
                for j in range(NC):
                    nc.tensor.matmul(psL1[:], h0T[:, j, :], w1x_sb[:, j, :],
                                     start=False, stop=(j == NC - 1))

                # ---- L1 cell ----
                h1_new, tgc1 = cell_elementwise(psL1, tgc1, "1")

                # ---- sync #2: gather h1(t) ----
                h1T_new = sync(h1_new, "h1T")

                # overlaps AG#2: next L0 Whh (h0(t))
                if t + 1 < T:
                    for j in range(NC):
                        nc.tensor.matmul(psL0n[:], h0T[:, j, :], w0h_sb[:, j, :],
                                         start=False, stop=False)

                h1T = h1T_new
                # ---- critical-path tail: next L0 composed-pred part ----
                if t + 1 < T:
                    for j in range(NC):
                        nc.tensor.matmul(psL0n[:], h1T[:, j, :], wc_sb[:, j, :],
                                         start=False, stop=(j == NC - 1))

                # ---- FC head (output only; off the recurrence) ----
                psFC = psC.tile([B, OUT], F32, tag="psFC")
                nc.tensor.matmul(psFC[:], ones_sb[:], bfc_sb[:], start=True, stop=False)
                for j in range(NC):
                    nc.tensor.matmul(psFC[:], h1T[:, j, :], wfc_sb[:, j, :],
                                     start=False, stop=(j == NC - 1))
                if t % 4 == 0:
                    pred_sb = spool.tile([B, 4, OUT], F32R, tag="pred")
                nc.vector.tensor_copy(pred_sb[:, t % 4, :], psFC[:])
                if t % 4 == 3 or t == T - 1:
                    lo = t - (t % 4)
                    nc.sync.dma_start(
                        outp[lo:t + 1].rearrange("t b o -> b t o"),
                        pred_sb[:, 0:t % 4 + 1, :])

                if t + 1 < T:
                    psL0 = psL0n

            # ---- final states ----
            nc.sync.dma_start(h_out[0], h0T[:])
            nc.sync.dma_start(h_out[1], h1T[:])
            nc.sync.dma_start(c_out[0], tgc0[:, :, 1, :])
            nc.sync.dma_start(c_out[1], tgc1[:, :, 1, :])

    nc.compile()
    return nc


# ----------------------------------------------------------------------------
# host-side sharding / unsharding
# ----------------------------------------------------------------------------

def _gate_rows(j):
    """Per-core gate row indices of one cell's [4*LH] gate dim, reordered to
    [i, f, o, g] (PyTorch layout is [i, f, g, o])."""
    base = j * S + np.arange(S)
    return np.concatenate([0 * LH + base, 1 * LH + base, 3 * LH + base, 2 * LH + base])


def _jmajor(h_major):
    """Permute an h-contraction [2*LH, N] matrix whose rows are ordered
    [dir0 units 0..511, dir1 units 0..511] into the gathered j-major row
    order (row j*128 + d*64 + i <-> dir d, unit j*64+i)."""
    r = np.arange(2 * LH)
    j, rem = divmod(r, 128)
    d, i = divmod(rem, S)
    idx = d * LH + j * S + i
    return h_major[idx]


def _scale_g(w):
    """Scale the g-gate columns (last S of each direction block) by 2."""
    w = w.copy()
    for d in range(2):
        w[..., d * G + 3 * S:d * G + 4 * S] *= 2
    return w


def make_in_maps(input_seq, W_ih, W_hh, b_ih, b_hh, W_fc, b_fc, T=T_FULL):
    input_seq = np.asarray(input_seq, np.float32)
    W_ih, W_hh = np.asarray(W_ih, np.float32), np.asarray(W_hh, np.float32)
    b_ih, b_hh = np.asarray(b_ih, np.float32), np.asarray(b_hh, np.float32)
    W_fc, b_fc = np.asarray(W_fc, np.float32), np.asarray(b_fc, np.float32)

    CDT = np.float16
    xT = np.ascontiguousarray(input_seq[:, :T, :].transpose(1, 2, 0)).astype(CDT)
    eye = np.eye(32, dtype=np.float32)
    ones = np.ones((1, B), CDT)
    wfc = np.ascontiguousarray(_jmajor(W_fc.T)).astype(CDT)  # [1024, 512], j-major
    W_fc64 = W_fc.astype(np.float64)
    b_fc64 = b_fc.astype(np.float64)

    in_maps = []
    for j in range(NC):
        rows = _gate_rows(j)
        w0x = np.concatenate([W_ih[0, d][rows][:, :IN].T for d in range(2)], axis=1)

        def hh(l):
            # per-direction block-diagonal Whh, rows h-major then j-major
            w = np.zeros((2 * LH, 2 * G), np.float32)
            for d in range(2):
                w[d * LH:(d + 1) * LH, d * G:(d + 1) * G] = W_hh[l, d][rows].T
            return np.ascontiguousarray(_jmajor(w))

        # composed pred feedback: (W0p_slice @ W_fc) over h1
        wc = np.concatenate(
            [(W_ih[0, d][rows][:, IN:].astype(np.float64) @ W_fc64).T.astype(np.float32)
             for d in range(2)], axis=1)
        w1x = np.concatenate([W_ih[1, d][rows].T for d in range(2)], axis=1)
        bias0 = np.concatenate(
            [(b_ih[0, d] + b_hh[0, d])[rows]
             + (W_ih[0, d][rows][:, IN:].astype(np.float64) @ b_fc64).astype(np.float32)
             for d in range(2)])
        bias0z = np.concatenate(
            [(b_ih[0, d] + b_hh[0, d])[rows] for d in range(2)])
        bias1 = np.concatenate([(b_ih[1, d] + b_hh[1, d])[rows] for d in range(2)])
        in_maps.append({
            "xT": xT,
            "w0x": _scale_g(np.ascontiguousarray(w0x)).astype(CDT),
            "w0h": _scale_g(hh(0)).astype(CDT),
            "wcomb": _scale_g(np.ascontiguousarray(_jmajor(wc))).astype(CDT),
            "w1x": _scale_g(np.ascontiguousarray(_jmajor(w1x))).astype(CDT),
            "w1h": _scale_g(hh(1)).astype(CDT),
            "wfc": wfc,
            "bias0": _scale_g(bias0.reshape(1, -1)).astype(CDT),
            "bias0z": _scale_g(bias0z.reshape(1, -1)).astype(CDT),
            "bias1": _scale_g(bias1.reshape(1, -1)).astype(CDT),
            "bfc": b_fc.reshape(1, OUT).astype(CDT), "eye": eye, "ones": ones,
            "zeros": np.zeros((128, NC, B), CDT),
        })
    return in_maps


def unshard(results, T=T_FULL):
    outputs = results[0]["outp"].transpose(1, 0, 2).astype(np.float32)  # [B,T,OUT]
    hT = results[0]["h_out"]  # [2, 128(p = d*64+i), NC(j), B]
    h_n = np.empty((4, B, LH), np.float32)
    for l in range(2):
        for d in range(2):
            blk = hT[l, d * S:(d + 1) * S, :, :]          # [i, j, b]
            h_n[2 * l + d] = blk.transpose(1, 0, 2).reshape(LH, B).T
    c_n = np.empty((4, B, LH), np.float32)
    for j in range(NC):
        cj = results[j]["c_out"]  # [2, B, 2, S]
        for l in range(2):
            for d in range(2):
                c_n[2 * l + d][:, j * S:(j + 1) * S] = cj[l, :, d, :]
    return outputs, h_n, c_n


_CACHE = {}


def kernel(input_seq, input_lengths, W_ih, W_hh, b_ih, b_hh, W_fc, b_fc):
    T = input_seq.shape[1]
    if T not in _CACHE:
        _CACHE[T] = build(T)
    nc = _CACHE[T]
    in_maps = make_in_maps(input_seq, W_ih, W_hh, b_ih, b_hh, W_fc, b_fc, T=T)
    res = run_bass_kernel_spmd(nc, in_maps, core_ids=list(range(NC)))
    return unshard(res.results, T=T)
